# revision 14
# baseline (speedup 1.0000x reference)
"""Trainium2 Bass kernel for the ActorNetwork GNN problem (self-contained).

Strategy
--------
The batched graph is identical for every batch element (the reference's
"offset trick"), so the normalized adjacency P = D^-1/2 (A+I) D^-1/2
[5000 x 5000] is shared across all 16 batch elements and both GCN layers.
Per-edge gather/scatter is hostile to Trainium (descriptor-rate bound), so
the aggregation is done as a *dense* matmul with P sharded by destination
node across the 8 cores: each core holds a [5120 x 640] bf16 slice of P^T
(SBUF-resident, built on the host from edge_index) and aggregates for all
16 batch elements at once (256-wide). The hidden features H [5120, 256]
(tiny) are exchanged with an AllGather between layers.

Everything is node-sharded: core c owns true nodes [c*625, (c+1)*625),
padded to 640 (= 5 x 128). Global padded node id = c*640 + local.
"""

import numpy as np
import ml_dtypes

BF16NP = ml_dtypes.bfloat16
FP8NP = ml_dtypes.float8_e4m3

B, N, F, E, C, FC = 16, 5000, 512, 160000, 64, 128
NCORES = 8
NLOC = N // NCORES            # 625 true nodes per core
NPAD = 640                    # padded nodes per core (5 x 128)
NT = NPAD // 128              # node tiles per core
NG = NCORES * NPAD            # 5120 padded global nodes
KT = NG // 128                # 40 src k-tiles
HF = 16                       # hidden feature width
GB = 8                        # batch elements per partition group
NGRP = B // GB                # 2 groups
BFW = B * HF                  # 256 = (batch, feat) width
FKT = F // 128                # 4 k-tiles for the input features

_GRAPH_CACHE = {}


# --------------------------------------------------------------------------
# Host-side preprocessing (index/layout work only)
# --------------------------------------------------------------------------

def _preprocess(inputs):
    nf = np.asarray(inputs["node_features"], dtype=np.float32)   # [B, N, F]
    cf = np.asarray(inputs["col_features"], dtype=np.float32)    # [B, C, FC]
    ei = np.asarray(inputs["edge_index"])                        # [2, E] int64

    src = ei[0].astype(np.int64)
    dst = ei[1].astype(np.int64)

    # Degrees / normalization exactly as the reference (in-degree + self loop)
    deg = np.bincount(dst, minlength=N).astype(np.float64) + 1.0
    dinv = 1.0 / np.sqrt(deg)
    norm = (dinv[src] * dinv[dst]).astype(np.float32)

    # Dense P^T [src_padded_global, dst_padded_global], f32 accumulate
    pg = lambda n: (n // NLOC) * NPAD + (n % NLOC)
    PT = np.zeros((NG, NG), dtype=np.float32)
    np.add.at(PT, (pg(src), pg(dst)), norm)
    loop = np.arange(N, dtype=np.int64)
    pl = pg(loop)
    PT[pl, pl] += (dinv * dinv).astype(np.float32)

    # P^T slices, p-major for contiguous DMA: [128, KT*NPAD] fp8.
    pt_cores = [
        np.ascontiguousarray(
            PT[:, c * NPAD:(c + 1) * NPAD].astype(FP8NP)
            .reshape(KT, 128, NPAD).transpose(1, 0, 2)
            .reshape(128, KT * NPAD))
        for c in range(NCORES)
    ]

    # X^T slices, p-major for contiguous DMA: [B, 128, FKT*NPAD] fp8.
    # Row p holds X^T[k*128+p, :] for k = 0..FKT-1 concatenated.
    xt_cores = []
    for c in range(NCORES):
        xt = np.zeros((B, F, NPAD), dtype=FP8NP)
        xt[:, :, :NLOC] = nf[:, c * NLOC:(c + 1) * NLOC, :].transpose(0, 2, 1)
        xt = np.ascontiguousarray(
            xt.reshape(B, FKT, 128, NPAD).transpose(0, 2, 1, 3)
            .reshape(B, 128, FKT * NPAD))
        xt_cores.append(xt)

    # Column features transposed: [FC, B*C] bf16 (replicated)
    cft = np.ascontiguousarray(
        cf.transpose(2, 0, 1).reshape(FC, B * C)).astype(BF16NP)

    W1 = np.asarray(inputs["W1"], np.float32)
    W2 = np.asarray(inputs["W2"], np.float32)
    fc_w = np.asarray(inputs["fc_w"], np.float32)
    fc_b = np.asarray(inputs["fc_b"], np.float32)
    cw1 = np.asarray(inputs["cw1"], np.float32)
    cb1 = np.asarray(inputs["cb1"], np.float32)
    cw2 = np.asarray(inputs["cw2"], np.float32)
    cb2 = np.asarray(inputs["cb2"], np.float32)
    b1 = np.asarray(inputs["b1"], np.float32)
    b2 = np.asarray(inputs["b2"], np.float32)

    shared = {
        "cft": cft,
        "w1": W1.astype(FP8NP),
        "wblk": np.kron(np.eye(GB, dtype=np.float32), W2).astype(BF16NP),
        "fcrep": np.kron(np.eye(GB, dtype=np.float32), fc_w).astype(BF16NP),
        "cw1": cw1.astype(BF16NP),
        "cw2": cw2.astype(BF16NP),
        "b1t": np.tile(b1, GB)[:, None].astype(np.float32),
        "b2t": np.tile(b2, GB)[:, None].astype(np.float32),
        "cb1": cb1[:, None].astype(np.float32),
        "clb": np.array([[fc_b[0] + cb2[0]]], dtype=np.float32),
    }
    return xt_cores, pt_cores, shared


# --------------------------------------------------------------------------
# Device graph (identical on all 8 cores)
# --------------------------------------------------------------------------

def _build_graph():
    from concourse import bacc
    import concourse.mybir as mybir
    import concourse.tile as tile
    from concourse.bass import ts

    f32 = mybir.dt.float32
    bf16 = mybir.dt.bfloat16
    fp8 = mybir.dt.float8e4
    AF = mybir.ActivationFunctionType
    DR = mybir.MatmulPerfMode.DoubleRow
    GW = GB * HF          # 128 = per-group (b,f) width
    KT2 = KT // 2         # 20 paired src k-tiles
    FKT2 = FKT // 2       # 2 paired feature k-tiles

    nc = bacc.Bacc("TRN2", target_bir_lowering=False, debug=False,
                   num_devices=NCORES)

    xt_e = nc.dram_tensor("xt", [B, 128, FKT * NPAD], fp8,
                          kind="ExternalInput")
    pt_e = nc.dram_tensor("pt", [128, KT * NPAD], fp8, kind="ExternalInput")
    cft_e = nc.dram_tensor("cft", [FC, B * C], bf16, kind="ExternalInput")
    w1_e = nc.dram_tensor("w1", [F, HF], fp8, kind="ExternalInput")
    wblk_e = nc.dram_tensor("wblk", [128, 128], bf16, kind="ExternalInput")
    fcrep_e = nc.dram_tensor("fcrep", [128, GB], bf16, kind="ExternalInput")
    cw1_e = nc.dram_tensor("cw1", [FC, HF], bf16, kind="ExternalInput")
    cw2_e = nc.dram_tensor("cw2", [HF, 1], bf16, kind="ExternalInput")
    b1_e = nc.dram_tensor("b1t", [128, 1], f32, kind="ExternalInput")
    b2_e = nc.dram_tensor("b2t", [128, 1], f32, kind="ExternalInput")
    cb1_e = nc.dram_tensor("cb1", [HF, 1], f32, kind="ExternalInput")
    clb_e = nc.dram_tensor("clb", [1, 1], f32, kind="ExternalInput")
    out_e = nc.dram_tensor("out", [B, 128, NT * C], f32,
                           kind="ExternalOutput")

    rg = [list(range(NCORES))]

    with tile.TileContext(nc) as tc:
        with (
            tc.tile_pool(name="const", bufs=1) as constp,
            tc.tile_pool(name="ptp", bufs=1) as ptp,
            tc.tile_pool(name="hallp", bufs=1) as hallp,
            tc.tile_pool(name="rowsp", bufs=1) as rowsp,
            tc.tile_pool(name="xlp", bufs=1) as xlp,
            tc.tile_pool(name="xsp", bufs=8) as xsp,
            tc.tile_pool(name="stgp", bufs=16) as stgp,
            tc.tile_pool(name="stg2p", bufs=4) as stg2p,
            tc.tile_pool(name="dram", bufs=1, space="DRAM") as dramp,
            tc.tile_pool(name="ps", bufs=2, space="PSUM") as ps,
        ):
            # ---- critical-path DMAs first: W1, then X, then P^T
            w1_sb = constp.tile([128, FKT, HF], fp8, name="w1_sb")
            nc.sync.dma_start(out=w1_sb[:],
                              in_=w1_e[:].rearrange("(k p) f -> p k f", p=128))
            x_tiles = []
            for b in range(B):
                x_t = xsp.tile([128, FKT, NPAD], fp8, tag="xt",
                               name=f"x_{b}")
                nc.sync.dma_start(
                    out=x_t[:].rearrange("p k n -> p (k n)"), in_=xt_e[b])
                x_tiles.append(x_t)
            pt_sb = ptp.tile([128, KT, NPAD], fp8, name="pt_sb")

            # ---- remaining constants
            wblk_sb = constp.tile([128, 128], bf16, name="wblk_sb")
            nc.sync.dma_start(out=wblk_sb[:], in_=wblk_e[:])
            fcrep_sb = constp.tile([128, GB], bf16, name="fcrep_sb")
            nc.sync.dma_start(out=fcrep_sb[:], in_=fcrep_e[:])
            cw1_sb = constp.tile([FC, HF], bf16, name="cw1_sb")
            nc.sync.dma_start(out=cw1_sb[:], in_=cw1_e[:])
            cw2_sb = constp.tile([HF, 1], bf16, name="cw2_sb")
            nc.sync.dma_start(out=cw2_sb[:], in_=cw2_e[:])
            b1_sb = constp.tile([128, 1], f32, name="b1_sb")
            nc.sync.dma_start(out=b1_sb[:], in_=b1_e[:])
            b2_sb = constp.tile([128, 1], f32, name="b2_sb")
            nc.sync.dma_start(out=b2_sb[:], in_=b2_e[:])
            cb1_sb = constp.tile([HF, 1], f32, name="cb1_sb")
            nc.sync.dma_start(out=cb1_sb[:], in_=cb1_e[:])
            clb_sb = constp.tile([1, 1], f32, name="clb_sb")
            nc.sync.dma_start(out=clb_sb[:], in_=clb_e[:])
            cft_sb = constp.tile([FC, B * C], bf16, name="cft_sb")
            nc.sync.dma_start(out=cft_sb[:], in_=cft_e[:])
            ones_sb = constp.tile([1, 128], bf16, name="ones_sb")
            nc.vector.memset(ones_sb[:], 1.0)

            def ag_pair(layer, g, src_tile):
                ag_in = dramp.tile([NPAD, GW], fp8, name=f"ag_in{layer}_{g}")
                ag_out = dramp.tile([NG, GW], fp8, addr_space="Shared",
                                    name=f"ag_out{layer}_{g}")
                nc.gpsimd.dma_start(
                    out=ag_in[:].rearrange("(t p) f -> p t f", p=128),
                    in_=src_tile[:])
                nc.gpsimd.collective_compute(
                    "AllGather",
                    mybir.AluOpType.bypass,
                    replica_groups=rg,
                    ins=[ag_in[:].opt()],
                    outs=[ag_out[:].opt()],
                )
                h_all = hallp.tile([128, KT, GW], fp8,
                                   tag=f"hall{g}", name=f"hall{layer}_{g}")
                for q in range(2):
                    nc.sync.dma_start(
                        out=h_all[:, ts(q, KT // 2), :],
                        in_=ag_out[q * (NG // 2):(q + 1) * (NG // 2), :]
                        .rearrange("(t p) f -> p t f", p=128))
                return h_all

            def ag_full(src_tiles):
                # one full-width AllGather for layer 1 (AG cost is mostly
                # size-independent here, so fewer collectives win)
                ag_in = dramp.tile([NPAD, BFW], fp8, name="ag_in0")
                ag_out = dramp.tile([NG, BFW], fp8, addr_space="Shared",
                                    name="ag_out0")
                for g in range(NGRP):
                    nc.gpsimd.dma_start(
                        out=ag_in[:].rearrange("(t p) f -> p t f", p=128)
                        [:, :, ts(g, GW)],
                        in_=src_tiles[g][:])
                nc.gpsimd.collective_compute(
                    "AllGather",
                    mybir.AluOpType.bypass,
                    replica_groups=rg,
                    ins=[ag_in[:].opt()],
                    outs=[ag_out[:].opt()],
                )
                h_all = hallp.tile([128, KT, BFW], fp8, tag="hallfull",
                                   name="hall0")
                for q in range(4):
                    nc.sync.dma_start(
                        out=h_all[:, ts(q, KT // 4), :],
                        in_=ag_out[q * (NG // 4):(q + 1) * (NG // 4), :]
                        .rearrange("(t p) f -> p t f", p=128))
                return h_all

            # ---- matmul1 (fp8 DoubleRow, node-major) + per-group AllGather
            h1g_tiles = []
            for g in range(NGRP):
                hg = rowsp.tile([128, NT, GW], fp8, tag=f"h1g{g}",
                                name=f"h1g_{g}")
                h1g_tiles.append(hg)
                for j in range(GB):
                    b = g * GB + j
                    mp = ps.tile([128, NT * HF], f32, tag="mm1b", bufs=3,
                                 name=f"mm1_{b}")
                    for t in range(NT):
                        for k2 in range(FKT2):
                            nc.tensor.matmul(
                                mp[:, ts(t, HF)],
                                lhsT=x_tiles[b][:, ts(k2, 2), ts(t, 128)],
                                rhs=w1_sb[:, ts(k2, 2), :],
                                perf_mode=DR,
                                start=(k2 == 0), stop=(k2 == FKT2 - 1))
                    nc.scalar.copy(
                        out=hg[:, :, ts(j, HF)],
                        in_=mp[:].rearrange("p (t f) -> p t f", t=NT))
            h1_full = ag_full(h1g_tiles)

            # ---- P^T loads: deferred so X has full HBM bandwidth during mm1
            for q in range(4):
                nc.sync.dma_start(
                    out=pt_sb[:, ts(q, KT // 4), :]
                    .rearrange("p t d -> p (t d)"),
                    in_=pt_e[:, q * (KT // 4) * NPAD:
                             (q + 1) * (KT // 4) * NPAD])

            # ---- PE filler during the first AllGather: column MLP + the
            # cl-broadcast matmuls of the joint head (independent of the GCN).
            colp = ps.tile([HF, B * C], f32, tag="big", bufs=2, name="colp")
            for h in range(2):
                nc.tensor.matmul(colp[:, ts(h, 512)], lhsT=cw1_sb[:],
                                 rhs=cft_sb[:, ts(h, 512)],
                                 start=True, stop=True)
            hcol_sb = constp.tile([HF, B * C], bf16, name="hcol_sb")
            nc.scalar.activation(out=hcol_sb[:], in_=colp[:], func=AF.Relu,
                                 bias=cb1_sb[:, 0:1])
            clp = ps.tile([1, B * C], f32, tag="big", bufs=2, name="clp")
            for h in range(2):
                nc.tensor.matmul(clp[:, ts(h, 512)], lhsT=cw2_sb[:],
                                 rhs=hcol_sb[:, ts(h, 512)],
                                 start=True, stop=True)
            cl_sb = constp.tile([1, B * C], bf16, name="cl_sb")
            nc.scalar.activation(out=cl_sb[:], in_=clp[:], func=AF.Identity,
                                 bias=clb_sb[:, 0:1])

            stages = {}

            def emit_jp(bs):
                for b in bs:
                    stage = stgp.tile([128, NT, C], f32, tag="stage",
                                      name=f"stage_{b}")
                    jp = ps.tile([128, NT * C], f32, tag="mm1b", bufs=3,
                                 name=f"jp_{b}")
                    for t in range(NT):
                        nc.tensor.matmul(jp[:, ts(t, C)], lhsT=ones_sb[:],
                                         rhs=cl_sb[0:1, ts(b, C)],
                                         start=True, stop=True)
                    nc.scalar.copy(
                        out=stage[:],
                        in_=jp[:].rearrange("p (t c) -> p t c", t=NT))
                    stages[b] = stage

            # PE warm-keeper: dummy matmuls executed in FIFO order fill the
            # collective-wait holes so HAM keeps the clock at 2.4 GHz.
            def keep_warm(n, tag_):
                dp = ps.tile([128, 16], f32, tag="dummy", bufs=1,
                             name=f"warm_{tag_}")
                for i in range(n):
                    nc.tensor.matmul(dp[:], lhsT=ones_sb[:],
                                     rhs=cl_sb[0:1, 0:16],
                                     start=True, stop=True)

            emit_jp(range(16))

            # ---- two GCN layers: dense aggregation (fp8 DR, 5x128-col)
            def agg_layer0(h_src):
                # Layer-1 aggregation, strip-outer: each 128-wide dst strip
                # finishes its full k-chain early, so relu -> mm2 -> h2 evac
                # -> bounce pipeline per strip and the layer-2 AllGather
                # triggers right after the last strip instead of a full
                # serial epilogue.
                for g in range(NGRP):
                    ap_ = ps.tile([128, NPAD], f32, tag="big", bufs=2,
                                  name=f"agg0_{g}")
                    x_g = xlp.tile([128, NPAD], bf16, tag=f"xl{g}",
                                   name=f"xl0_{g}")
                    h2 = rowsp.tile([128, NT, GW], fp8, tag=f"h2g{g}",
                                    name=f"h2g_{g}")
                    ag_in = dramp.tile([NPAD, GW], fp8, name=f"ag_in1_{g}")
                    ag_out = dramp.tile([NG, GW], fp8, addr_space="Shared",
                                        name=f"ag_out1_{g}")
                    for t in range(NT):
                        for k2 in range(KT2):
                            nc.tensor.matmul(
                                ap_[:, ts(t, 128)],
                                lhsT=h_src[:, ts(k2, 2), ts(g, GW)],
                                rhs=pt_sb[:, ts(k2, 2), ts(t, 128)],
                                perf_mode=DR,
                                start=(k2 == 0), stop=(k2 == KT2 - 1))
                        nc.scalar.activation(out=x_g[:, ts(t, 128)],
                                             in_=ap_[:, ts(t, 128)],
                                             func=AF.Relu,
                                             bias=b1_sb[:, 0:1])
                        mp2 = ps.tile([128, 128], f32, tag="mm1b", bufs=3,
                                      name=f"mm2_{g}_{t}")
                        nc.tensor.matmul(mp2[:],
                                         lhsT=x_g[:, ts(t, 128)],
                                         rhs=wblk_sb[:],
                                         start=True, stop=True)
                        nc.scalar.copy(out=h2[:, t, :], in_=mp2[:])
                        nc.gpsimd.dma_start(
                            out=ag_in[:].rearrange("(t p) f -> p t f", p=128)
                            [:, t, :],
                            in_=h2[:, t, :])
                    nc.gpsimd.collective_compute(
                        "AllGather",
                        mybir.AluOpType.bypass,
                        replica_groups=rg,
                        ins=[ag_in[:].opt()],
                        outs=[ag_out[:].opt()],
                    )
                    h_all = hallp.tile([128, KT, GW], fp8, tag=f"hall{g}",
                                       name=f"hall1_{g}")
                    for q in range(2):
                        nc.sync.dma_start(
                            out=h_all[:, ts(q, KT // 2), :],
                            in_=ag_out[q * (NG // 2):(q + 1) * (NG // 2), :]
                            .rearrange("(t p) f -> p t f", p=128))
                    h2_all[g] = h_all

            h2_all = [None, None]
            agg_layer0(h1_full)
            keep_warm(128, "b")

            # layer 2 aggregation + head per group (finish g0 fully so its
            # output DMAs overlap g1's aggregation)
            bias2 = b2_sb
            for g in range(NGRP):
                ap_ = ps.tile([128, NPAD], f32, tag="big", bufs=2,
                              name=f"agg1_{g}")
                for k2 in range(KT2):
                    lhs = h2_all[g][:, ts(k2, 2), :]
                    nc.tensor.matmul(
                        ap_[:, 0:512], lhsT=lhs,
                        rhs=pt_sb[:, ts(k2, 2), 0:512],
                        perf_mode=DR,
                        start=(k2 == 0), stop=(k2 == KT2 - 1))
                    nc.tensor.matmul(
                        ap_[:, 512:NPAD], lhsT=lhs,
                        rhs=pt_sb[:, ts(k2, 2), 512:NPAD],
                        perf_mode=DR,
                        start=(k2 == 0), stop=(k2 == KT2 - 1))
                x_g = xlp.tile([128, NPAD], bf16, tag=f"xl{g}",
                               name=f"xl2_{g}")
                nc.scalar.activation(out=x_g[:], in_=ap_[:], func=AF.Relu,
                                     bias=bias2[:, 0:1])

                # node logits for this group
                nlt = rowsp.tile([128, NT, GB], f32, tag=f"nl{g}",
                                 name=f"nl_{g}")
                np_ = ps.tile([128, NT * GB], f32, tag="mm1b", bufs=3,
                              name=f"nlp_{g}")
                for t in range(NT):
                    nc.tensor.matmul(np_[:, ts(t, GB)],
                                     lhsT=x_g[:, ts(t, 128)],
                                     rhs=fcrep_sb[:], start=True, stop=True)
                nc.scalar.copy(out=nlt[:],
                               in_=np_[:].rearrange("p (t f) -> p t f", t=NT))

                # joint add + output for this group's batches
                for j in range(GB):
                    b = g * GB + j
                    st2 = stg2p.tile([128, NT, C], f32, tag="stage2",
                                     name=f"st2_{b}")
                    nc.vector.tensor_add(
                        out=st2[:],
                        in0=stages[b][:],
                        in1=nlt[:, :, j:j + 1].to_broadcast([128, NT, C]))
                    nc.sync.dma_start(
                        out=out_e[b],
                        in_=st2[:].rearrange("p t c -> p (t c)"))

    nc.compile()
    return nc


def _get_graph():
    if "nc" not in _GRAPH_CACHE:
        _GRAPH_CACHE["nc"] = _build_graph()
    return _GRAPH_CACHE["nc"]


# --------------------------------------------------------------------------
# Entry point
# --------------------------------------------------------------------------

def _run(inputs, trace=False, tmpdir=None):
    from concourse.bass_utils import run_bass_kernel_spmd

    xt_cores, pt_cores, shared = _preprocess(inputs)
    nc = _get_graph()
    in_maps = []
    for c in range(NCORES):
        m = dict(shared)
        m["xt"] = xt_cores[c]
        m["pt"] = pt_cores[c]
        in_maps.append(m)
    res = run_bass_kernel_spmd(nc, in_maps, core_ids=list(range(NCORES)),
                               trace=trace, tmpdir=tmpdir)
    out = np.zeros((B, N, C), dtype=np.float32)
    for c in range(NCORES):
        o = np.asarray(res.results[c]["out"])            # [B, 128, NT*C]
        o = o.reshape(B, 128, NT, C).transpose(0, 2, 1, 3).reshape(B, NPAD, C)
        out[:, c * NLOC:(c + 1) * NLOC, :] = o[:, :NLOC, :]
    return out.reshape(B, N * C), res


def kernel(**inputs) -> np.ndarray:
    out, _ = _run(inputs, trace=False)
    return out
